# revision 28
# baseline (speedup 1.0000x reference)
"""MultiHeadAttention Trainium2 kernel, 8-way tensor-parallel over heads.

B=4, T=2048, C=1024, H=16 heads, Dh=64. Each of the 8 NeuronCores owns 2
heads: it computes Q^T/K^T (as [2*Dh, T] per batch), V (as [T, 2*Dh] via PE
transpose), attention in the transposed layout (S^T = K_tile^T @ Q^T with the
contraction over Dh; exp on ScalarE; P@V as V_ones^T @ E^T which also yields
the softmax denominator from an appended ones column), and a partial output
projection against its 128 columns of W_out. The host sums the 8 partials
(fp64) to unshard.

All matmuls run as float32r (full-rate fp32, ~1e-4 relative accuracy).
"""
import sys
sys.path.insert(0, '/opt/trn_rl_repo')
import numpy as np

import concourse.bass as bass
import concourse.mybir as mybir
import concourse.tile as tile
from concourse import bacc
from concourse.bass_utils import run_bass_kernel_spmd
from concourse.masks import make_identity

F32 = mybir.dt.float32
F32R = mybir.dt.float32r
AF = mybir.ActivationFunctionType

B, T, C = 4, 2048, 1024
H, DH = 16, 64
NCORES = 8
HPC = H // NCORES          # heads per core (2)
D2 = HPC * DH              # 128, local concat dim
BT = B * T                 # 8192
NT = T // 512              # q/t tiles of 512 per batch (4)
NK = T // 128              # k tiles of 128 per batch (16)
CCH = C // 128             # contraction chunks (8)

_NC_CACHE = {}


def build_nc():
    nc = bacc.Bacc()

    xp = nc.dram_tensor("xp", [128, B * NT, CCH, 512], F32, kind="ExternalInput")
    wq = nc.dram_tensor("wq", [128, CCH, D2], F32, kind="ExternalInput")
    wk = nc.dram_tensor("wk", [128, CCH, D2], F32, kind="ExternalInput")
    wv = nc.dram_tensor("wv", [128, CCH, D2], F32, kind="ExternalInput")
    bq = nc.dram_tensor("bq", [D2, 1], F32, kind="ExternalInput")
    bk = nc.dram_tensor("bk", [D2, 1], F32, kind="ExternalInput")
    bv = nc.dram_tensor("bv", [D2, 1], F32, kind="ExternalInput")
    wo = nc.dram_tensor("wo", [128, C], F32, kind="ExternalInput")
    bo = nc.dram_tensor("bo", [128, C], F32, kind="ExternalInput")
    y = nc.dram_tensor("y", [BT, C], F32, kind="ExternalOutput")

    with tile.TileContext(nc) as tc:
        with (
            tc.tile_pool(name="singles", bufs=1) as singles,
            tc.tile_pool(name="xin", bufs=3) as xin,
            tc.tile_pool(name="qkv", bufs=2) as qkv,
            tc.tile_pool(name="vtmp", bufs=2) as vtmp_pool,
            tc.tile_pool(name="esb", bufs=4) as esb,
            tc.tile_pool(name="rsb", bufs=2) as rsb,
            tc.tile_pool(name="osb", bufs=2) as osb,
            tc.tile_pool(name="outsb", bufs=3) as outsb,
            # 8 PSUM banks total: s2 2x2 + pv 2x1 + po 2x1
            tc.tile_pool(name="s2_ps", bufs=2, space="PSUM") as s2_ps,
            tc.tile_pool(name="small_ps", bufs=4, space="PSUM") as small_ps,
        ):
            ident = singles.tile([128, 128], F32)
            make_identity(nc, ident)
            warm_f = singles.tile([128, 512], F32, tag="warm_f")
            nc.vector.memset(warm_f, 1.0)
            warm_r = singles.tile([128, 512], F32R, tag="warm_r")
            nc.vector.tensor_copy(out=warm_r, in_=warm_f)
            for wi in range(12):
                wps = small_ps.tile([128, 512], F32, tag="sm", name=f"warm{wi}")
                nc.tensor.matmul(out=wps, lhsT=warm_r[:, 0:128], rhs=warm_r,
                                 start=True, stop=True)
            ones16 = singles.tile([128, NK, 1], F32)
            nc.vector.memset(ones16, 1.0)

            wq_sb = singles.tile([128, CCH, D2], F32R, tag="wq")
            wk_sb = singles.tile([128, CCH, D2], F32R, tag="wk")
            wv_sb = singles.tile([128, CCH, D2], F32R, tag="wv")
            for w_dram, w_sb in ((wq, wq_sb), (wk, wk_sb), (wv, wv_sb)):
                nc.sync.dma_start(out=w_sb, in_=w_dram[:, :, :].bitcast(F32R))
            bq_sb = singles.tile([D2, 1], F32, tag="bq")
            bk_sb = singles.tile([D2, 1], F32, tag="bk")
            bv_sb = singles.tile([D2, 1], F32, tag="bv")
            nc.sync.dma_start(out=bq_sb, in_=bq[:, :])
            nc.sync.dma_start(out=bk_sb, in_=bk[:, :])
            nc.sync.dma_start(out=bv_sb, in_=bv[:, :])
            wo_sb = singles.tile([128, C], F32R, tag="wo")
            nc.sync.dma_start(out=wo_sb, in_=wo[:, :].bitcast(F32R))
            bo_sb = singles.tile([128, C], F32, tag="bo")
            nc.sync.dma_start(out=bo_sb, in_=bo[:, :])

            for b in range(B):
                qT = qkv.tile([D2, T], F32R, tag="q")
                kT = qkv.tile([D2, T], F32R, tag="k")
                # per k-tile lhsT layout (193 cols):
                #   h0: cols 0:65   = [V_h0 | 1]            (M=65:  num@0:64, Z@64)
                #   h1: cols 65:193 = [junk32 | 1 | junk31 | V_h1] (M=128: Z@32,
                #        num@64:128; junk columns make junk PSUM rows, never read)
                v1 = qkv.tile([128, NK, 193], F32R, tag="v")
                nc.vector.tensor_copy(out=v1[:, :, DH:DH + 1], in_=ones16)
                nc.vector.tensor_copy(out=v1[:, :, 97:98], in_=ones16)

                # ---- QKV projection for batch b ----
                for tt in range(NT):
                    t0 = tt * 512
                    xt = xin.tile([128, CCH, 512], F32R)
                    nc.sync.dma_start(
                        out=xt,
                        in_=xp[:, b * NT + tt, :, :].bitcast(F32R))
                    for w_sb, b_sb, dest in ((wq_sb, bq_sb, qT), (wk_sb, bk_sb, kT)):
                        ps = small_ps.tile([128, 512], F32, tag="sm")
                        for ci in range(CCH):
                            nc.tensor.matmul(out=ps, lhsT=w_sb[:, ci, :],
                                             rhs=xt[:, ci, :],
                                             start=(ci == 0), stop=(ci == CCH - 1))
                        nc.vector.tensor_scalar_add(out=dest[:, t0:t0 + 512],
                                                    in0=ps, scalar1=b_sb)
                    # V^T, then transpose into [t, d] layout
                    ps = small_ps.tile([128, 512], F32, tag="sm")
                    for ci in range(CCH):
                        nc.tensor.matmul(out=ps, lhsT=wv_sb[:, ci, :],
                                         rhs=xt[:, ci, :],
                                         start=(ci == 0), stop=(ci == CCH - 1))
                    vt = vtmp_pool.tile([128, 512], F32)
                    nc.vector.tensor_scalar_add(out=vt, in0=ps, scalar1=bv_sb)
                    for s in range(4):
                        tp = small_ps.tile([128, 512], F32, tag="sm")
                        nc.tensor.transpose(out=tp[:, 0:128],
                                            in_=vt[:, s * 128:(s + 1) * 128],
                                            identity=ident)
                        kt = tt * 4 + s
                        sl = v1[:, kt, :]
                        dst = bass.AP(tensor=sl.tensor, offset=sl.offset,
                                      ap=[list(sl.ap[0]), [129, 2], [1, DH]])
                        nc.vector.tensor_copy(
                            out=dst,
                            in_=tp[:, 0:128].rearrange("p (g x) -> p g x", g=2))

                # ---- attention for batch b (both heads interleaved; the
                # S(kt) matmuls are issued before PV(kt-1) so the in-order PE
                # queue never stalls behind the exp of the current kt) ----
                oT2 = osb.tile([128, T], F32R, tag="o2")
                for qt in range(NT):
                    q0 = qt * 512
                    pv0 = small_ps.tile([DH + 1, 512], F32, tag="sm")
                    pv1 = small_ps.tile([128, 512], F32, tag="sm")
                    pvs = [pv0, pv1]
                    lh = [(0, DH + 1), (DH + 1, 193)]
                    ets = []
                    LA = 2  # PV lookahead: PV(kt-LA) issues with S(kt)
                    for kt in range(NK):
                        s2 = s2_ps.tile([128, 1024], F32, tag="s2")
                        for h in range(HPC):
                            hs = h * DH
                            nc.tensor.matmul(
                                out=s2[:, h * 512:(h + 1) * 512],
                                lhsT=kT[hs:hs + DH, kt * 128:(kt + 1) * 128],
                                rhs=qT[hs:hs + DH, q0:q0 + 512],
                                start=True, stop=True)
                        if kt >= LA:
                            etp = ets[kt - LA]
                            for h in range(HPC):
                                nc.tensor.matmul(
                                    out=pvs[h],
                                    lhsT=v1[:, kt - LA, lh[h][0]:lh[h][1]],
                                    rhs=etp[:, h * 512:(h + 1) * 512],
                                    start=(kt - LA == 0), stop=False)
                        et = esb.tile([128, 1024], F32R)
                        nc.scalar.activation(out=et, in_=s2,
                                             func=AF.Exp, scale=0.125)
                        ets.append(et)
                    for kt in range(NK - LA, NK):
                        for h in range(HPC):
                            nc.tensor.matmul(
                                out=pvs[h],
                                lhsT=v1[:, kt, lh[h][0]:lh[h][1]],
                                rhs=ets[kt][:, h * 512:(h + 1) * 512],
                                start=False, stop=(kt == NK - 1))
                    # evacuate each PV accumulator to SBUF with one copy (frees
                    # the PSUM slot immediately); normalize from the copy.
                    # h0: num@[0:64], Z@[64]; h1: num@[64:128], Z@[32]
                    for h, pv, zrow, o_lo, o_hi in (
                            (0, pv0, DH, 0, DH),
                            (1, pv1, 32, DH, 128)):
                        pvc = rsb.tile([128, 512], F32, tag="pvc")
                        if h == 0:
                            nc.vector.tensor_copy(out=pvc[0:DH + 1, :],
                                                  in_=pv[0:DH + 1, :])
                        else:
                            nc.vector.tensor_copy(out=pvc[32:33, :],
                                                  in_=pv[32:33, :])
                            nc.vector.tensor_copy(out=pvc[DH:128, :],
                                                  in_=pv[DH:128, :])
                        z = rsb.tile([1, 512], F32, tag="z")
                        nc.vector.tensor_copy(out=z, in_=pvc[zrow:zrow + 1, :])
                        r = rsb.tile([1, 512], F32, tag="r")
                        nc.vector.reciprocal_approx_fast(out=r, in_=z)
                        rbc = rsb.tile([128, 512], F32, tag="rbc")
                        nc.gpsimd.partition_broadcast(rbc[0:o_hi, :], r)
                        nc.vector.tensor_mul(out=oT2[o_lo:o_hi, q0:q0 + 512],
                                             in0=pvc[o_lo:o_hi, :],
                                             in1=rbc[o_lo:o_hi, :])

                # ---- partial out-projection for batch b ----
                for ts in range(T // 128):
                    ot = outsb.tile([128, C], F32)
                    for n in range(2):
                        n0 = n * 512
                        ops = small_ps.tile([128, 512], F32, tag="sm")
                        nc.tensor.matmul(
                            out=ops,
                            lhsT=oT2[:, ts * 128:(ts + 1) * 128],
                            rhs=wo_sb[:, n0:n0 + 512],
                            start=True, stop=True)
                        nc.vector.tensor_add(out=ot[:, n0:n0 + 512],
                                             in0=ops, in1=bo_sb[:, n0:n0 + 512])
                    nc.gpsimd.dma_start(
                        out=y[b * T + ts * 128:b * T + (ts + 1) * 128, :],
                        in_=ot)

    nc.compile()
    return nc


def make_in_maps(x, W_qkv, b_qkv, W_out, b_out):
    # x pre-tiled to the exact SBUF layout: xp[p, tile, ci, c] = x[tile*512+c, ci*128+p]
    xp = np.ascontiguousarray(
        x.reshape(B * NT, 512, CCH, 128).transpose(3, 0, 2, 1))
    bo = np.ascontiguousarray(
        np.broadcast_to(b_out.astype(np.float32) / NCORES, (128, C)))
    in_maps = []
    for c in range(NCORES):
        r0 = c * D2
        def wshuf(wslice):
            # [D2, C] weight rows -> lhsT chunks [128 p, CCH, D2]
            return np.ascontiguousarray(
                wslice.T.reshape(CCH, 128, D2).transpose(1, 0, 2))
        wq = wshuf(W_qkv[r0:r0 + D2, :])
        wk = wshuf(W_qkv[C + r0:C + r0 + D2, :])
        wv = wshuf(W_qkv[2 * C + r0:2 * C + r0 + D2, :])
        bqc = np.ascontiguousarray(b_qkv[r0:r0 + D2].reshape(D2, 1))
        bkc = np.ascontiguousarray(b_qkv[C + r0:C + r0 + D2].reshape(D2, 1))
        bvc = np.ascontiguousarray(b_qkv[2 * C + r0:2 * C + r0 + D2].reshape(D2, 1))
        woc = np.ascontiguousarray(W_out[:, r0:r0 + D2].T)
        in_maps.append({
            "xp": xp, "wq": wq, "wk": wk, "wv": wv,
            "bq": bqc, "bk": bkc, "bv": bvc, "wo": woc, "bo": bo,
        })
    return in_maps


def run(x, W_qkv, b_qkv, W_out, b_out, trace=False):
    if "nc" not in _NC_CACHE:
        _NC_CACHE["nc"] = build_nc()
    nc = _NC_CACHE["nc"]
    in_maps = make_in_maps(
        np.asarray(x, dtype=np.float32), np.asarray(W_qkv, dtype=np.float32),
        np.asarray(b_qkv, dtype=np.float32), np.asarray(W_out, dtype=np.float32),
        np.asarray(b_out, dtype=np.float32))
    res = run_bass_kernel_spmd(nc, in_maps, core_ids=list(range(NCORES)),
                               trace=trace)
    acc = np.zeros((BT, C), dtype=np.float64)
    for c in range(NCORES):
        acc += res.results[c]["y"]
    out = acc.astype(np.float32).reshape(B, T, C)
    return out, res


def kernel(x, W_qkv, b_qkv, W_out, b_out):
    out, _ = run(x, W_qkv, b_qkv, W_out, b_out, trace=False)
    return out
